# revision 1
# baseline (speedup 1.0000x reference)
"""Cross-attention-concat kernel for Trainium2 (8 NeuronCores, Bass/Tile).

Math (per batch b):
  x   = concat(rgb, chm) on channels           [512, 4096]   (pixels hw = h*64+w)
  Q   = Wq x + bq ; K = Wk x + bk              [64, ...]
  V   = Wv x + bv                              [256, 4096]
  S   = Q^T K                                  [2048 hw, 4096 xy]
  A   = softmax over y within each x-group of 64 keys
  out = Wcr (A V^T)^T + bcr                    [256, 2048]

Sharding: core = (batch, H-half). The host rolls each batch's pixel axis by
2048*(core%2) so every core runs the same program with its queries at
columns 0:2048 of the rolled image (attention is invariant to the roll:
K/V/attn permute together and the roll is a multiple of the y-group 64).

Precision: scores/softmax-denominator path is fp32 (exp amplifies input
error); the attention matrix and V are bf16 (PE streams bf16 at 1 col/cycle
vs 2 for fp32, with fp32 PSUM accumulation), final projection fp32.
"""

import numpy as np
import ml_dtypes

B, C, H, W = 4, 256, 64, 64
HW = H * W               # 4096
CIN = 2 * C              # 512
QCOLS = HW // 2          # 2048 queries per core
NSUP = QCOLS // 512      # 4 super-blocks of 512 queries (4 sub-blocks of 128)

_CACHE = {}


def _patch_ldw_opt():
    # the default compile pins --enable-ldw-opt=false; with one distinct
    # stationary per matmul the serialized weight loads cost ~100ns/matmul.
    import concourse.bass_utils as bu

    if getattr(bu, "_ldw_patched", False):
        return
    orig = bu.run_command

    # note: --enable-ldw-opt=true fails walrus codegen (visitInstLdweights);
    # keep the default flags.
    bu._ldw_patched = True
    del orig


def _build_nc():
    _patch_ldw_opt()
    import concourse.bacc as bacc
    import concourse.tile as tile
    from concourse import mybir
    from concourse.masks import make_identity

    F32 = mybir.dt.float32
    F32R = mybir.dt.float32r
    F16 = mybir.dt.float16
    BF16 = mybir.dt.bfloat16
    AX = mybir.AxisListType
    AF = mybir.ActivationFunctionType

    nc = bacc.Bacc("TRN2", target_bir_lowering=False, debug=False, num_devices=8)

    x_d = nc.dram_tensor("x", [CIN, HW], BF16, kind="ExternalInput").ap()
    g64_d = nc.dram_tensor("g64", [128, 32, 128], mybir.dt.bfloat16, kind="ExternalInput").ap()
    g128_d = nc.dram_tensor("g128", [128, 32, 128], mybir.dt.bfloat16, kind="ExternalInput").ap()
    wqk_d = nc.dram_tensor("wqk", [128, 4, 128], BF16, kind="ExternalInput").ap()
    wvt_d = nc.dram_tensor("wvt", [128, 4, 256], BF16, kind="ExternalInput").ap()
    wcr_d = nc.dram_tensor("wcr", [128, 2, 2, 128], BF16, kind="ExternalInput").ap()
    bq_d = nc.dram_tensor("bq2", [64, 1], F32, kind="ExternalInput").ap()
    bk_d = nc.dram_tensor("bk2", [64, 1], F32, kind="ExternalInput").ap()
    bv64_d = nc.dram_tensor("bv64", [128, 2], F32, kind="ExternalInput").ap()
    bcr_d = nc.dram_tensor("bcr2", [128, 2], F32, kind="ExternalInput").ap()
    zz_d = nc.dram_tensor("zz", [64, HW], mybir.dt.bfloat16, kind="ExternalInput").ap()
    out_d = nc.dram_tensor("out", [C, QCOLS], F32, kind="ExternalOutput").ap()

    with tile.TileContext(nc) as tc:
        with (
            tc.tile_pool(name="const", bufs=1) as constp,
            tc.tile_pool(name="qkv", bufs=1) as qkvp,
            tc.tile_pool(name="pbuf", bufs=40) as pbufp,
            tc.tile_pool(name="ptbuf", bufs=4) as ptp,
            tc.tile_pool(name="attbuf", bufs=2) as attbp,
            tc.tile_pool(name="dbuf", bufs=2) as dbufp,
            tc.tile_pool(name="obuf", bufs=2) as obufp,
        ):
            # ---- constants ----
            wqk_sb = constp.tile([128, 4, 128], BF16)
            wvt_sb = constp.tile([128, 4, 256], BF16)
            wcr_sb = constp.tile([128, 2, 2, 128], BF16)
            bq_sb = constp.tile([64, 1], F32)
            bk_sb = constp.tile([64, 1], F32)
            bv64_sb = constp.tile([128, 2], F32)
            bcr_sb = constp.tile([128, 2], F32)
            g64_sb = constp.tile([128, 32, 128], BF16)
            g128_sb = constp.tile([128, 32, 128], BF16)
            nc.sync.dma_start(out=wqk_sb, in_=wqk_d)
            nc.sync.dma_start(out=wvt_sb, in_=wvt_d)
            nc.sync.dma_start(out=wcr_sb, in_=wcr_d)
            nc.sync.dma_start(out=bq_sb, in_=bq_d)
            nc.sync.dma_start(out=bk_sb, in_=bk_d)
            nc.sync.dma_start(out=bv64_sb, in_=bv64_d)
            nc.sync.dma_start(out=bcr_sb, in_=bcr_d)
            nc.sync.dma_start(out=g64_sb, in_=g64_d)
            nc.sync.dma_start(out=g128_sb, in_=g128_d)

            q_sb = qkvp.tile([128, QCOLS], BF16)      # Q for this core's queries
            k_sb = qkvp.tile([128, HW], BF16)         # K, full image
            vt_sb = qkvp.tile([128, 32, 256], BF16)  # V^T, [xy-block, 128, 256]

            nc.sync.dma_start(out=q_sb[64:128, :], in_=zz_d[:, 0:QCOLS])
            nc.sync.dma_start(out=k_sb[64:128, :], in_=zz_d)

            # ---- preamble: load x, compute Q, K, V^T ----
            with tc.tile_pool(name="xp", bufs=1) as xp, \
                 tc.tile_pool(name="ps_pre", bufs=4, space="PSUM") as ps_pre, \
                 tc.tile_pool(name="ps_prek", bufs=2, space="PSUM") as ps_prek:
                x_sb = []
                for k in range(4):
                    xk = xp.tile([128, HW], BF16, tag=f"x{k}", name=f"x{k}")
                    for j in range(4):
                        sl = slice(j * 1024, (j + 1) * 1024)
                        nc.sync.dma_start(out=xk[:, sl], in_=x_d[k * 128 : (k + 1) * 128, sl])
                    x_sb.append(xk)

                # Q over this core's 2048 query columns (fp32)
                for n in range(4):
                    psq = ps_pre.tile([64, 512], F32, tag="pre", name="psq")
                    for k in range(4):
                        nc.tensor.matmul(
                            psq,
                            lhsT=wqk_sb[:, k, 0:64],
                            rhs=x_sb[k][:, n * 512 : (n + 1) * 512],
                            start=(k == 0),
                            stop=(k == 3),
                        )
                    nc.scalar.add(q_sb[0:64, n * 512 : (n + 1) * 512], psq, bq_sb)
                # K over the full image (fp32), 1024-col pairs
                for n in range(4):
                    psk = ps_prek.tile([64, 1024], F32, tag="prek", name="psk")
                    for j in range(2):
                        for k in range(4):
                            nc.tensor.matmul(
                                psk[:, j * 512 : (j + 1) * 512],
                                lhsT=wqk_sb[:, k, 64:128],
                                rhs=x_sb[k][:, (2 * n + j) * 512 : (2 * n + j + 1) * 512],
                                start=(k == 0),
                                stop=(k == 3),
                            )
                    nc.vector.tensor_scalar_add(
                        k_sb[0:64, n * 1024 : (n + 1) * 1024], psk, bk_sb
                    )
                # V^T (bf16 inputs, fp32 psum, bf16 out): out [xy 128, c 256]
                for i2 in range(16):
                    psv = ps_pre.tile([128, 512], F32, tag="pre", name="psv")
                    for j in range(2):
                        i = 2 * i2 + j
                        for k in range(4):
                            nc.tensor.matmul(
                                psv[:, j * 256 : (j + 1) * 256],
                                lhsT=x_sb[k][:, i * 128 : (i + 1) * 128],
                                rhs=wvt_sb[:, k, :],
                                start=(k == 0),
                                stop=(k == 3),
                            )
                    dst = vt_sb[:, 2 * i2 : 2 * i2 + 2, :]
                    if i2 % 2 == 0:
                        nc.scalar.copy(dst, psv)
                    else:
                        nc.vector.tensor_copy(dst, psv)

            # ---- main loop over super-blocks of 512 queries ----
            # Transposed-scores formulation: compute S^T [xy, q] directly
            # (lhsT = K block, rhs = Q block), exp in that layout, reduce
            # the per-x-group softmax denominators with a tiny constant
            # G2 matmul (partition-group sums), broadcast 1/d back with a
            # G2b matmul into PSUM, and fuse the scaling into the single
            # PSUM->SBUF pass whose output feeds A@V. No PE transposes,
            # no vector reduce, no separate scale pass.
            NI = HW // 128  # 32 xy-blocks
            with (
                tc.tile_pool(name="ps_sc", bufs=2, space="PSUM") as ps_sc,
                tc.tile_pool(name="ps_d", bufs=1, space="PSUM") as ps_d,
                tc.tile_pool(name="ps_bc", bufs=2, space="PSUM") as ps_bc,
                tc.tile_pool(name="ps_att", bufs=1, space="PSUM") as ps_att,
                tc.tile_pool(name="ps_fin", bufs=1, space="PSUM") as ps_fin,
            ):
                e_tiles = {}   # (S, i) -> exp(S^T) tile, bf16 [128, 512]
                d_ps = {}      # S -> PSUM [64, 512] per-x-group sums
                rd_sb = {}     # S -> SBUF bf16 [64, 512] reciprocal sums
                att_h = {}     # S -> accumulating PSUM pair
                scale_live = {}

                def emit_scores(S, i):
                    q0 = S * 512
                    pssc = ps_sc.tile([128, 512], F32, tag="sc", name="pssc")
                    nc.tensor.matmul(
                        pssc,
                        lhsT=k_sb[:, i * 128 : (i + 1) * 128],
                        rhs=q_sb[:, q0 : q0 + 512],
                        start=True,
                        stop=True,
                    )
                    e = pbufp.tile([128, 512], BF16, tag="e", name="e")
                    nc.scalar.activation(e, pssc, AF.Exp)
                    e_tiles[(S, i)] = e

                def emit_dsum(S, i):
                    if S not in d_ps:
                        d_ps[S] = ps_d.tile([128, 512], F32, tag="d", name="d_ps")
                    nc.tensor.matmul(
                        d_ps[S],
                        lhsT=g64_sb[:, i, :],
                        rhs=e_tiles[(S, i)],
                        start=(i == 0),
                        stop=(i == NI - 1),
                    )

                def emit_recip(S):
                    rd = dbufp.tile([128, 512], BF16, tag="rd", name="rd")
                    with nc.allow_low_precision(reason="1/d broadcast operand is bf16 by design"):
                        nc.vector.reciprocal(rd, d_ps[S])
                    rd_sb[S] = rd

                def emit_bcav(S, i):
                    if S not in att_h:
                        att_h[S] = [
                            ps_att.tile([128, 512], F32, tag=f"att{h}", name=f"att{h}")
                            for h in range(2)
                        ]
                    scale_ps = ps_bc.tile([128, 512], F32, tag="bc", name="scale_ps")
                    nc.tensor.matmul(
                        scale_ps,
                        lhsT=g128_sb[:, i, :],
                        rhs=rd_sb[S],
                        start=True,
                        stop=True,
                    )
                    es = ptp.tile([128, 512], BF16, tag="es", name="es")
                    with nc.allow_low_precision(reason="attn weights are bf16 by design"):
                        nc.vector.tensor_mul(es, e_tiles[(S, i)], scale_ps)
                    for h in range(2):
                        nc.tensor.matmul(
                            att_h[S][h],
                            lhsT=vt_sb[:, i, h * 128 : (h + 1) * 128],
                            rhs=es,
                            start=(i == 0),
                            stop=(i == NI - 1),
                        )

                def emit_final(S):
                    ah = att_h.pop(S)
                    attT = attbp.tile([128, 2, 512], BF16, tag="attT", name="attT")
                    nc.scalar.add(attT[:, 0, :], ah[0], bv64_sb[:, 0:1])
                    nc.scalar.add(attT[:, 1, :], ah[1], bv64_sb[:, 1:2])
                    out_t = obufp.tile([128, 2, 512], F32, tag="out_t", name="out_t")
                    for g in range(2):
                        psf = ps_fin.tile([128, 512], F32, tag="fin", name="psf")
                        for h in range(2):
                            nc.tensor.matmul(
                                psf,
                                lhsT=wcr_sb[:, h, g, :],
                                rhs=attT[:, h, :],
                                start=(h == 0),
                                stop=(h == 1),
                            )
                        if g == 0:
                            nc.scalar.add(out_t[:, g, :], psf, bcr_sb[:, g : g + 1])
                        else:
                            nc.scalar.add(out_t[:, g, :], psf, bcr_sb[:, g : g + 1])
                        nc.sync.dma_start(
                            out=out_d[g * 128 : (g + 1) * 128, S * 512 : (S + 1) * 512],
                            in_=out_t[:, g, :],
                        )

                # Software pipeline over the in-order PE stream: per i,
                # interleave scores(S) with the broadcast+AV of S-1, and
                # trail the d-sums of S two iterations behind their exp.
                prev = None
                for S in range(NSUP):
                    for i in range(NI):
                        emit_scores(S, i)
                        if prev is not None:
                            emit_bcav(prev, i)
                        if i >= 2:
                            emit_dsum(S, i - 2)
                    if prev is not None:
                        emit_final(prev)
                    emit_dsum(S, NI - 2)
                    emit_dsum(S, NI - 1)
                    emit_recip(S)
                    prev = S
                for i in range(NI):
                    emit_bcav(prev, i)
                emit_final(prev)
    nc.compile()
    return nc


def get_nc():
    if "nc" not in _CACHE:
        _CACHE["nc"] = _build_nc()
    return _CACHE["nc"]


def make_in_maps(inputs):
    rgb = np.asarray(inputs["rgb_features"], np.float32)
    chm = np.asarray(inputs["chm_features"], np.float32)
    Wq = np.asarray(inputs["Wq"], np.float32)
    bq = np.asarray(inputs["bq"], np.float32)
    Wk = np.asarray(inputs["Wk"], np.float32)
    bk = np.asarray(inputs["bk"], np.float32)
    Wv = np.asarray(inputs["Wv"], np.float32)
    bv = np.asarray(inputs["bv"], np.float32)
    Wcr = np.asarray(inputs["Wcr"], np.float32)
    bcr = np.asarray(inputs["bcr"], np.float32)

    Wqk = np.concatenate([Wq, Wk], axis=0)  # [128, 512]
    wqk = np.ascontiguousarray(Wqk.T.reshape(4, 128, 128).transpose(1, 0, 2)).astype(ml_dtypes.bfloat16)
    wvt = np.ascontiguousarray(Wv.T.reshape(4, 128, 256).transpose(1, 0, 2)).astype(ml_dtypes.bfloat16)
    wcr = np.ascontiguousarray(Wcr.T.reshape(2, 128, 2, 128).transpose(1, 0, 2, 3)).astype(ml_dtypes.bfloat16)
    bq2 = np.ascontiguousarray(bq.reshape(64, 1))
    bk2 = np.ascontiguousarray(bk.reshape(64, 1))
    bv64 = np.ascontiguousarray((64.0 * bv).reshape(2, 128).T)
    bcr2 = np.ascontiguousarray(bcr.reshape(2, 128).T)

    # selector constants for the softmax denominators:
    # g64[p, i, c] = 1 iff c == 2i + p//64 (scatter block i's two
    # 64-partition-half sums into rows 2i, 2i+1 of the d accumulator)
    # g128[k, i, p] = 1 iff k == 2i + p//64 (broadcast rows 2i, 2i+1 of
    # 1/d back across the two partition halves of block i)
    p_ix = np.arange(128)
    g64 = np.zeros((128, 32, 128), ml_dtypes.bfloat16)
    g128 = np.zeros((128, 32, 128), ml_dtypes.bfloat16)
    for i in range(32):
        # cols 64:128 duplicate cols 0:64 so the padded d rows hold finite
        # sums (their reciprocal is then killed by g128's zero rows)
        g64[p_ix, i, 2 * i + p_ix // 64] = 1
        g64[p_ix, i, 64 + 2 * i + p_ix // 64] = 1
        g128[2 * i + p_ix // 64, i, p_ix] = 1

    in_maps = []
    for core in range(8):
        b, par = divmod(core, 2)
        x = np.concatenate([rgb[b], chm[b]], axis=0).reshape(CIN, HW)
        if par:
            x = np.roll(x, -QCOLS, axis=1)
        x = np.ascontiguousarray(x)
        in_maps.append(
            {
                "x": x.astype(ml_dtypes.bfloat16),
                "wqk": wqk,
                "wvt": wvt,
                "wcr": wcr,
                "bq2": bq2,
                "bk2": bk2,
                "bv64": bv64,
                "bcr2": bcr2,
                "g64": g64,
                "zz": np.zeros((64, HW), ml_dtypes.bfloat16),
                "g128": g128,
            }
        )
    return in_maps


def assemble(outs):
    full = np.empty((B, C, HW), np.float32)
    for core in range(8):
        b, par = divmod(core, 2)
        full[b, :, par * QCOLS : (par + 1) * QCOLS] = outs[core]
    return full.reshape(B, C, H, W)


def kernel(**inputs):
    from concourse.bass_utils import run_bass_kernel_spmd

    nc = get_nc()
    res = run_bass_kernel_spmd(nc, make_in_maps(inputs), core_ids=list(range(8)))
    return assemble([r["out"] for r in res.results])



# revision 4
# speedup vs baseline: 1.0787x; 1.0787x over previous
"""Cross-attention-concat kernel for Trainium2 (8 NeuronCores, Bass/Tile).

Math (per batch b):
  x   = concat(rgb, chm) on channels           [512, 4096]   (pixels hw = h*64+w)
  Q   = Wq x + bq ; K = Wk x + bk              [64, ...]
  V   = Wv x + bv                              [256, 4096]
  S   = Q^T K                                  [2048 hw, 4096 xy]
  A   = softmax over y within each x-group of 64 keys
  out = Wcr (A V^T)^T + bcr                    [256, 2048]

Sharding: core = (batch, H-half). The host rolls each batch's pixel axis by
2048*(core%2) so every core runs the same program with its queries at
columns 0:2048 of the rolled image.

Key structure (v2):
- Transposed-scores formulation: S^T [xy, q] per 128-xy block, exp on
  ScalarE, per-x-group softmax denominators collected by a selector matmul
  (g64), reciprocal broadcast back by a second selector matmul (g128),
  scale fused into the PSUM->SBUF pass feeding A@V.
- The two K=64-contraction matmuls (scores: c=64; 1/d broadcast: rd rows)
  are ROW-PACKED: two concurrent matmuls on PE row-halves via
  tile_position, halving their stream time. K blocks are packed
  even/odd into partition halves (k2), Q is duplicated into both halves,
  rd carries d duplicated into both halves (via g64's dup columns).
- Preamble is DMA-pipelined column-chunk-wise with a fused [Wq|Wk]
  stationary; Q/K land via small SBUF->SBUF shuffle DMAs.
- Reciprocal uses the fast approx DVE op and is emitted one pipeline
  stage early so it never stalls the PE.
"""

import numpy as np
import ml_dtypes

B, C, H, W = 4, 256, 64, 64
HW = H * W               # 4096
CIN = 2 * C              # 512
QCOLS = HW // 2          # 2048 queries per core
NSUP = QCOLS // 512      # 4 super-blocks of 512 queries
NI = HW // 128           # 32 xy-blocks of 128
NP = NI // 2             # 16 block-pairs

_CACHE = {}


def _build_nc():
    import concourse.bacc as bacc
    import concourse.tile as tile
    from concourse import mybir

    F32 = mybir.dt.float32
    BF16 = mybir.dt.bfloat16
    AF = mybir.ActivationFunctionType

    nc = bacc.Bacc("TRN2", target_bir_lowering=False, debug=False, num_devices=8)

    x_d = nc.dram_tensor("x", [CIN, HW], BF16, kind="ExternalInput").ap()
    g64_d = nc.dram_tensor("g64b", [128, 190], BF16, kind="ExternalInput").ap()
    g128_d = nc.dram_tensor("g128p", [128, 16, 128], BF16, kind="ExternalInput").ap()
    wqk_d = nc.dram_tensor("wqk", [128, 4, 128], BF16, kind="ExternalInput").ap()
    wvt_d = nc.dram_tensor("wvt", [128, 4, 256], BF16, kind="ExternalInput").ap()
    wcr_d = nc.dram_tensor("wcr", [128, 2, 2, 128], BF16, kind="ExternalInput").ap()
    bqk_d = nc.dram_tensor("bqk", [128, 1], F32, kind="ExternalInput").ap()
    bv64_d = nc.dram_tensor("bv64", [128, 2], F32, kind="ExternalInput").ap()
    bcr_d = nc.dram_tensor("bcr2", [128, 2], F32, kind="ExternalInput").ap()
    out_d = nc.dram_tensor("out", [C, QCOLS], F32, kind="ExternalOutput").ap()

    with tile.TileContext(nc) as tc:
        with (
            tc.tile_pool(name="const", bufs=1) as constp,
            tc.tile_pool(name="qkv", bufs=1) as qkvp,
            tc.tile_pool(name="pbuf", bufs=40) as pbufp,
            tc.tile_pool(name="esbuf", bufs=2) as esp,
            tc.tile_pool(name="rdbuf", bufs=2) as rdp,
            tc.tile_pool(name="attbuf", bufs=2) as attbp,
            tc.tile_pool(name="obuf", bufs=2) as obufp,
        ):
            # ---- constants (weights first; big selector consts after x) ----
            wqk_sb = constp.tile([128, 4, 128], BF16)
            bqk_sb = constp.tile([128, 1], F32)
            nc.sync.dma_start(out=wqk_sb, in_=wqk_d)
            nc.sync.dma_start(out=bqk_sb, in_=bqk_d)

            q_sb = qkvp.tile([128, QCOLS], BF16)       # Q dup'd in both halves
            k2_sb = qkvp.tile([128, NP, 128], BF16)    # K blocks even/odd packed
            vt_sb = qkvp.tile([128, NI, 256], BF16)    # V^T [xy-block, 128, 256]

            wvt_sb = constp.tile([128, 4, 256], BF16)
            wcr_sb = constp.tile([128, 2, 2, 128], BF16)
            bv64_sb = constp.tile([128, 2], F32)
            bcr_sb = constp.tile([128, 2], F32)
            g64_sb = constp.tile([128, 190], BF16)
            g128_sb = constp.tile([128, 16, 128], BF16)

            # ---- preamble: pipelined over 8 column-chunks of 512 ----
            with tc.tile_pool(name="xp", bufs=1) as xp, \
                 tc.tile_pool(name="ps_pre", bufs=2, space="PSUM") as ps_pre:
                x_sb = [xp.tile([128, HW], BF16, tag=f"x{k}", name=f"x{k}") for k in range(4)]
                for j in range(8):
                    for k in range(4):
                        sl = slice(j * 512, (j + 1) * 512)
                        nc.sync.dma_start(out=x_sb[k][:, sl], in_=x_d[k * 128:(k + 1) * 128, sl])
                # weights the preamble/main needs later: fill DMA idle time
                nc.sync.dma_start(out=wvt_sb, in_=wvt_d)
                nc.sync.dma_start(out=wcr_sb, in_=wcr_d)
                nc.sync.dma_start(out=bv64_sb, in_=bv64_d)
                nc.sync.dma_start(out=bcr_sb, in_=bcr_d)
                nc.sync.dma_start(out=g64_sb, in_=g64_d)
                nc.sync.dma_start(out=g128_sb, in_=g128_d)

                # qkb[:, j, a, b, :] : col 512j+256a+128b; rows 0:64 = Q, 64:128 = K
                qkb = xp.tile([128, 8, 2, 2, 128], BF16, name="qkb")
                for j in range(8):
                    sl = slice(j * 512, (j + 1) * 512)
                    psqk = ps_pre.tile([128, 512], F32, tag="psqk", name="psqk")
                    for k in range(4):
                        nc.tensor.matmul(
                            psqk,
                            lhsT=wqk_sb[:, k, :],
                            rhs=x_sb[k][:, sl],
                            start=(k == 0),
                            stop=(k == 3),
                        )
                    if j % 2 == 0:
                        nc.scalar.add(qkb[:, j], psqk, bqk_sb)
                    else:
                        nc.vector.tensor_scalar_add(qkb[:, j], psqk, bqk_sb)
                    # K shuffle: even xy-blocks -> window0, odd -> window1
                    nc.sync.dma_start(out=k2_sb[0:64, 2 * j:2 * j + 2, :], in_=qkb[64:128, j, :, 0, :])
                    nc.sync.dma_start(out=k2_sb[64:128, 2 * j:2 * j + 2, :], in_=qkb[64:128, j, :, 1, :])
                    if j < 4:
                        nc.sync.dma_start(out=q_sb[0:64, j * 512:(j + 1) * 512], in_=qkb[0:64, j])
                        nc.sync.dma_start(out=q_sb[64:128, j * 512:(j + 1) * 512], in_=qkb[0:64, j])
                    # V^T for this chunk's 4 xy-blocks (x is the stationary)
                    for i2v in range(2):
                        psv = ps_pre.tile([128, 512], F32, tag="psv", name="psv")
                        for bb in range(2):
                            i = 4 * j + 2 * i2v + bb
                            for k in range(4):
                                nc.tensor.matmul(
                                    psv[:, bb * 256:(bb + 1) * 256],
                                    lhsT=x_sb[k][:, i * 128:(i + 1) * 128],
                                    rhs=wvt_sb[:, k, :],
                                    start=(k == 0),
                                    stop=(k == 3),
                                )
                        dst = vt_sb[:, 4 * j + 2 * i2v:4 * j + 2 * i2v + 2, :]
                        if i2v == 0:
                            nc.scalar.copy(dst, psv)
                        else:
                            nc.vector.tensor_copy(dst, psv)

            # ---- main loop ----
            with (
                tc.tile_pool(name="ps_sc", bufs=1, space="PSUM") as ps_sc,
                tc.tile_pool(name="ps_d", bufs=1, space="PSUM") as ps_d,
                tc.tile_pool(name="ps_bc", bufs=1, space="PSUM") as ps_bc,
                tc.tile_pool(name="ps_att", bufs=1, space="PSUM") as ps_att,
                tc.tile_pool(name="ps_fin", bufs=1, space="PSUM") as ps_fin,
            ):
                e_tiles = {}   # (S, i) -> exp(S^T) tile, bf16 [128, 512]
                d_ps = {}      # S -> PSUM [128, 512] per-x-group sums (dup'd halves)
                rd_sb = {}     # S -> SBUF bf16 [128, 512] reciprocal sums
                att_h = {}     # S -> accumulating PSUM pair

                def emit_scores(S, p):
                    # pair p -> xy-blocks (2p, 2p+1) on PE row-halves
                    q0 = S * 512
                    for w in range(2):
                        ps = ps_sc.tile([128, 512], F32, tag=f"sc{w}", name=f"sc{w}")
                        nc.tensor.matmul(
                            ps,
                            lhsT=k2_sb[64 * w:64 * w + 64, p, :],
                            rhs=q_sb[64 * w:64 * w + 64, q0:q0 + 512],
                            start=True,
                            stop=True,
                        )
                        e = pbufp.tile([128, 512], BF16, tag="e", name="e")
                        nc.scalar.activation(e, ps, AF.Exp)
                        e_tiles[(S, 2 * p + w)] = e

                def emit_dsum(S, p):
                    # blocks (2p, 2p+1): scatter per-half sums into d rows
                    if S not in d_ps:
                        d_ps[S] = ps_d.tile([128, 512], F32, tag="d", name="d_ps")
                    for i in (2 * p, 2 * p + 1):
                        nc.tensor.matmul(
                            d_ps[S],
                            lhsT=g64_sb[:, 62 - 2 * i:190 - 2 * i],
                            rhs=e_tiles[(S, i)],
                            start=(i == 0),
                            stop=(i == NI - 1),
                        )

                def emit_recip(S):
                    rd32 = rdp.tile([128, 512], F32, tag="rd32", name="rd32")
                    nc.vector.reciprocal_approx_fast(out=rd32, in_=d_ps.pop(S))
                    rd = rdp.tile([128, 512], BF16, tag="rd", name="rd")
                    nc.vector.tensor_copy(rd, rd32)
                    rd_sb[S] = rd

                def emit_final(S):
                    ah = att_h.pop(S)
                    attT = attbp.tile([128, 2, 512], BF16, tag="attT", name="attT")
                    nc.scalar.add(attT[:, 0, :], ah[0], bv64_sb[:, 0:1])
                    nc.scalar.add(attT[:, 1, :], ah[1], bv64_sb[:, 1:2])
                    out_t = obufp.tile([128, 2, 512], F32, tag="out_t", name="out_t")
                    for g in range(2):
                        psf = ps_fin.tile([128, 512], F32, tag="fin", name="psf")
                        for h in range(2):
                            nc.tensor.matmul(
                                psf,
                                lhsT=wcr_sb[:, h, g, :],
                                rhs=attT[:, h, :],
                                start=(h == 0),
                                stop=(h == 1),
                            )
                        nc.scalar.add(out_t[:, g, :], psf, bcr_sb[:, g:g + 1])
                        nc.sync.dma_start(
                            out=out_d[g * 128:(g + 1) * 128, S * 512:(S + 1) * 512],
                            in_=out_t[:, g, :],
                        )

                es_live = {}   # (S, t) -> es tile pair from emit_bc2

                def emit_bc2(S, t):
                    pair = []
                    for w, blk in ((0, t), (1, t + 16)):
                        ps = ps_bc.tile([128, 512], F32, tag=f"bc{w}", name=f"bc{w}")
                        nc.tensor.matmul(
                            ps,
                            lhsT=g128_sb[64 * w:64 * w + 64, t, :],
                            rhs=rd_sb[S][64 * w:64 * w + 64, :],
                            start=True,
                            stop=True,
                        )
                        es = esp.tile([128, 512], BF16, tag=f"es{w}", name=f"es{w}")
                        with nc.allow_low_precision(reason="attn weights are bf16 by design"):
                            nc.vector.tensor_mul(es, e_tiles[(S, blk)], ps)
                        pair.append(es)
                    es_live[(S, t)] = pair

                def emit_av2(S, t):
                    if S not in att_h:
                        att_h[S] = [
                            ps_att.tile([128, 512], F32, tag=f"att{h}", name=f"att{h}")
                            for h in range(2)
                        ]
                    pair = es_live.pop((S, t))
                    for w, blk in ((0, t), (1, t + 16)):
                        for h in range(2):
                            nc.tensor.matmul(
                                att_h[S][h],
                                lhsT=vt_sb[:, blk, h * 128:(h + 1) * 128],
                                rhs=pair[w],
                                start=(t == 0 and w == 0),
                                stop=(t == NP - 1 and w == 1),
                            )

                # Software pipeline: consumer side (bc/av/final of S-1) trails
                # the producer side (scores/dsum of S) far enough that the
                # reciprocal latency at each super-block boundary is covered.
                for S in range(NSUP + 1):
                    prod = S < NSUP
                    cons = S >= 1
                    for p in range(NP):
                        if prod:
                            emit_scores(S, p)
                            if p >= 1:
                                emit_dsum(S, p - 1)
                        if cons and p >= 1:
                            emit_bc2(S - 1, p - 1)
                        if cons and p >= 2:
                            emit_av2(S - 1, p - 2)
                    if prod:
                        emit_dsum(S, NP - 1)
                        emit_recip(S)
                    if cons:
                        emit_bc2(S - 1, NP - 1)
                        emit_av2(S - 1, NP - 2)
                        emit_av2(S - 1, NP - 1)
                        emit_final(S - 1)
    nc.compile()
    return nc


def get_nc():
    if "nc" not in _CACHE:
        _CACHE["nc"] = _build_nc()
    return _CACHE["nc"]


def make_in_maps(inputs):
    rgb = np.asarray(inputs["rgb_features"], np.float32)
    chm = np.asarray(inputs["chm_features"], np.float32)
    Wq = np.asarray(inputs["Wq"], np.float32)
    bq = np.asarray(inputs["bq"], np.float32)
    Wk = np.asarray(inputs["Wk"], np.float32)
    bk = np.asarray(inputs["bk"], np.float32)
    Wv = np.asarray(inputs["Wv"], np.float32)
    bv = np.asarray(inputs["bv"], np.float32)
    Wcr = np.asarray(inputs["Wcr"], np.float32)
    bcr = np.asarray(inputs["bcr"], np.float32)

    Wqk = np.concatenate([Wq, Wk], axis=0)  # [128, 512]
    wqk = np.ascontiguousarray(Wqk.T.reshape(4, 128, 128).transpose(1, 0, 2)).astype(ml_dtypes.bfloat16)
    wvt = np.ascontiguousarray(Wv.T.reshape(4, 128, 256).transpose(1, 0, 2)).astype(ml_dtypes.bfloat16)
    wcr = np.ascontiguousarray(Wcr.T.reshape(2, 128, 2, 128).transpose(1, 0, 2, 3)).astype(ml_dtypes.bfloat16)
    bqk = np.ascontiguousarray(np.concatenate([bq, bk]).reshape(128, 1))
    bv64 = np.ascontiguousarray((64.0 * bv).reshape(2, 128).T)
    bcr2 = np.ascontiguousarray(bcr.reshape(2, 128).T)

    # dsum selector, shift-base form: slice for block i is g64b[:, 62-2i:190-2i]
    # with ones at u = 62+p//64 (d rows 2i+p//64) and u = 126+p//64 (dup rows
    # 64+2i+p//64, keeping every d_ps row finite for the reciprocal).
    p_ix = np.arange(128)
    g64b = np.zeros((128, 190), ml_dtypes.bfloat16)
    g64b[p_ix, 62 + p_ix // 64] = 1
    g64b[p_ix, 126 + p_ix // 64] = 1

    # row-packed 1/d broadcast selectors: pair t -> blocks (t, t+16).
    # window0 (partitions 0:64) serves block t from rd rows 2t + m//64;
    # window1 (partitions 64:128) serves block t+16 from the dup rows.
    g128p = np.zeros((128, 16, 128), ml_dtypes.bfloat16)
    m_ix = np.arange(128)
    for t in range(16):
        g128p[2 * t + m_ix // 64, t, m_ix] = 1
        g128p[64 + 2 * t + 32 + m_ix // 64, t, m_ix] = 1

    in_maps = []
    for core in range(8):
        b, par = divmod(core, 2)
        x = np.concatenate([rgb[b], chm[b]], axis=0).reshape(CIN, HW)
        if par:
            x = np.roll(x, -QCOLS, axis=1)
        x = np.ascontiguousarray(x)
        in_maps.append(
            {
                "x": x.astype(ml_dtypes.bfloat16),
                "wqk": wqk,
                "wvt": wvt,
                "wcr": wcr,
                "bqk": bqk,
                "bv64": bv64,
                "bcr2": bcr2,
                "g64b": g64b,
                "g128p": g128p,
            }
        )
    return in_maps


def assemble(outs):
    full = np.empty((B, C, HW), np.float32)
    for core in range(8):
        b, par = divmod(core, 2)
        full[b, :, par * QCOLS:(par + 1) * QCOLS] = outs[core]
    return full.reshape(B, C, H, W)


def kernel(**inputs):
    from concourse.bass_utils import run_bass_kernel_spmd

    nc = get_nc()
    res = run_bass_kernel_spmd(nc, make_in_maps(inputs), core_ids=list(range(8)))
    return assemble([r["out"] for r in res.results])


# revision 10
# speedup vs baseline: 1.1162x; 1.0348x over previous
"""Cross-attention-concat kernel for Trainium2 (8 NeuronCores, Bass/Tile).

Math (per batch b):
  x   = concat(rgb, chm) on channels           [512, 4096]   (pixels hw = h*64+w)
  Q   = Wq x + bq ; K = Wk x + bk              [64, ...]
  V   = Wv x + bv                              [256, 4096]
  S   = Q^T K                                  [2048 hw, 4096 xy]
  A   = softmax over y within each x-group of 64 keys
  out = Wcr (A V^T)^T + bcr                    [256, 2048]

Sharding: core = (batch, H-half). The host rolls each batch's pixel axis by
2048*(core%2) so every core runs the same program with its queries at
columns 0:2048 of the rolled image.

Key structure (v2):
- Transposed-scores formulation: S^T [xy, q] per 128-xy block, exp on
  ScalarE, per-x-group softmax denominators collected by a selector matmul
  (g64), reciprocal broadcast back by a second selector matmul (g128),
  scale fused into the PSUM->SBUF pass feeding A@V.
- The two K=64-contraction matmuls (scores: c=64; 1/d broadcast: rd rows)
  are ROW-PACKED: two concurrent matmuls on PE row-halves via
  tile_position, halving their stream time. K blocks are packed
  even/odd into partition halves (k2), Q is duplicated into both halves,
  rd carries d duplicated into both halves (via g64's dup columns).
- Preamble is DMA-pipelined column-chunk-wise with a fused [Wq|Wk]
  stationary; Q/K land via small SBUF->SBUF shuffle DMAs.
- Reciprocal uses the fast approx DVE op and is emitted one pipeline
  stage early so it never stalls the PE.
"""

import numpy as np
import ml_dtypes

B, C, H, W = 4, 256, 64, 64
HW = H * W               # 4096
CIN = 2 * C              # 512
QCOLS = HW // 2          # 2048 queries per core
NSUP = QCOLS // 512      # 4 super-blocks of 512 queries
NI = HW // 128           # 32 xy-blocks of 128
NP = NI // 2             # 16 block-pairs

_CACHE = {}


def _build_nc():
    import concourse.bacc as bacc
    import concourse.tile as tile
    from concourse import mybir

    F32 = mybir.dt.float32
    BF16 = mybir.dt.bfloat16
    AF = mybir.ActivationFunctionType

    nc = bacc.Bacc("TRN2", target_bir_lowering=False, debug=False, num_devices=8)

    x_d = nc.dram_tensor("x", [CIN, HW], BF16, kind="ExternalInput").ap()
    g64_d = nc.dram_tensor("g64b", [128, 190], BF16, kind="ExternalInput").ap()
    g128_d = nc.dram_tensor("g128p", [128, 16, 128], BF16, kind="ExternalInput").ap()
    wqk_d = nc.dram_tensor("wqk", [128, 4, 128], BF16, kind="ExternalInput").ap()
    wvt_d = nc.dram_tensor("wvt", [128, 4, 256], BF16, kind="ExternalInput").ap()
    wcr_d = nc.dram_tensor("wcr", [128, 2, 2, 128], BF16, kind="ExternalInput").ap()
    bqk_d = nc.dram_tensor("bqk", [128, 1], F32, kind="ExternalInput").ap()
    bv64_d = nc.dram_tensor("bv64", [128, 2], F32, kind="ExternalInput").ap()
    bcr_d = nc.dram_tensor("bcr2", [128, 2], F32, kind="ExternalInput").ap()
    out_d = nc.dram_tensor("out", [C, QCOLS], BF16, kind="ExternalOutput").ap()

    with tile.TileContext(nc) as tc:
        with (
            tc.tile_pool(name="const", bufs=1) as constp,
            tc.tile_pool(name="qkv", bufs=1) as qkvp,
            tc.tile_pool(name="pbuf", bufs=40) as pbufp,
            tc.tile_pool(name="esbuf", bufs=2) as esp,
            tc.tile_pool(name="rdbuf", bufs=2) as rdp,
            tc.tile_pool(name="attbuf", bufs=2) as attbp,
            tc.tile_pool(name="obuf", bufs=2) as obufp,
        ):
            # ---- constants (weights first; big selector consts after x) ----
            wqk_sb = constp.tile([128, 4, 128], BF16)
            bqk_sb = constp.tile([128, 1], F32)
            nc.sync.dma_start(out=wqk_sb, in_=wqk_d)
            nc.sync.dma_start(out=bqk_sb, in_=bqk_d)

            q_sb = qkvp.tile([128, QCOLS], BF16)       # Q dup'd in both halves
            k2_sb = qkvp.tile([128, NP, 128], BF16)    # K blocks even/odd packed
            vt_sb = qkvp.tile([128, NI, 256], BF16)    # V^T [xy-block, 128, 256]

            wvt_sb = constp.tile([128, 4, 256], BF16)
            wcr_sb = constp.tile([128, 2, 2, 128], BF16)
            bv64_sb = constp.tile([128, 2], F32)
            bcr_sb = constp.tile([128, 2], F32)
            g64_sb = constp.tile([128, 190], BF16)
            g128_sb = constp.tile([128, 16, 128], BF16)

            # ---- preamble: 2 big x chunks, warm-up MMs during the DMA head,
            # QK projection only (V is interleaved into S0 below) ----
            x_sb = [qkvp.tile([128, HW], BF16, tag=f"x{k}", name=f"x{k}") for k in range(4)]
            # qkb[:, j, a, b, :] : col 512j+256a+128b; rows 0:64 = Q, 64:128 = K
            qkb = qkvp.tile([128, 8, 2, 2, 128], BF16, name="qkb")
            with tc.tile_pool(name="ps_pre", bufs=1, space="PSUM") as ps_pre:
                junk = ps_pre.tile([128, 512], F32, tag="junk", name="junk")
                for _ in range(10):
                    nc.tensor.matmul(junk, lhsT=wqk_sb[:, 0, :], rhs=wqk_sb[:, :, :], start=True, stop=True)
                for half in range(2):
                    for k in range(4):
                        sl = slice(half * 2048, (half + 1) * 2048)
                        nc.sync.dma_start(out=x_sb[k][:, sl], in_=x_d[k * 128:(k + 1) * 128, sl])
                nc.sync.dma_start(out=wvt_sb, in_=wvt_d)
                nc.sync.dma_start(out=g64_sb, in_=g64_d)
                nc.sync.dma_start(out=g128_sb, in_=g128_d)
                nc.sync.dma_start(out=wcr_sb, in_=wcr_d)
                nc.sync.dma_start(out=bv64_sb, in_=bv64_d)
                nc.sync.dma_start(out=bcr_sb, in_=bcr_d)

                for j in range(8):
                    sl = slice(j * 512, (j + 1) * 512)
                    psqk = ps_pre.tile([128, 512], F32, tag="psqk", bufs=2, name="psqk")
                    for k in range(4):
                        nc.tensor.matmul(
                            psqk,
                            lhsT=wqk_sb[:, k, :],
                            rhs=x_sb[k][:, sl],
                            start=(k == 0),
                            stop=(k == 3),
                        )
                    if j % 2 == 0:
                        nc.scalar.add(qkb[:, j], psqk, bqk_sb)
                    else:
                        nc.vector.tensor_scalar_add(qkb[:, j], psqk, bqk_sb)
                    if j == 3:
                        # Q dup: both partition halves get the same biased Q
                        nc.sync.dma_start(out=q_sb[0:64, :], in_=qkb[0:64, 0:4])
                        nc.sync.dma_start(out=q_sb[64:128, :], in_=qkb[0:64, 0:4])
                # K shuffle: even xy-blocks -> window0, odd -> window1
                nc.sync.dma_start(out=k2_sb[0:64, :, :], in_=qkb[64:128, :, :, 0, :])
                nc.sync.dma_start(out=k2_sb[64:128, :, :], in_=qkb[64:128, :, :, 1, :])

            # ---- main loop ----
            with (
                tc.tile_pool(name="ps_sc", bufs=1, space="PSUM") as ps_sc,
                tc.tile_pool(name="ps_d", bufs=1, space="PSUM") as ps_d,
                tc.tile_pool(name="ps_bc", bufs=1, space="PSUM") as ps_bc,
                tc.tile_pool(name="ps_att", bufs=1, space="PSUM") as ps_att,
                tc.tile_pool(name="ps_fin", bufs=1, space="PSUM") as ps_fin,
            ):
                e_tiles = {}   # (S, i) -> exp(S^T) tile, bf16 [128, 512]
                d_ps = {}      # S -> PSUM [128, 512] per-x-group sums (dup'd halves)
                rd_sb = {}     # S -> SBUF bf16 [128, 512] reciprocal sums
                att_h = {}     # S -> accumulating PSUM pair

                def emit_scores(S, p):
                    # pair p -> xy-blocks (2p, 2p+1) on PE row-halves
                    q0 = S * 512
                    for w in range(2):
                        ps = ps_sc.tile([128, 512], F32, tag=f"sc{w}", name=f"sc{w}")
                        nc.tensor.matmul(
                            ps,
                            lhsT=k2_sb[64 * w:64 * w + 64, p, :],
                            rhs=q_sb[64 * w:64 * w + 64, q0:q0 + 512],
                            start=True,
                            stop=True,
                        )
                        e = pbufp.tile([128, 512], BF16, tag="e", name="e")
                        nc.scalar.activation(e, ps, AF.Exp)
                        e_tiles[(S, 2 * p + w)] = e

                def emit_dsum(S, p):
                    # blocks (2p, 2p+1): scatter per-half sums into d rows
                    if S not in d_ps:
                        d_ps[S] = ps_d.tile([128, 512], F32, tag="d", name="d_ps")
                    for i in (2 * p, 2 * p + 1):
                        nc.tensor.matmul(
                            d_ps[S],
                            lhsT=g64_sb[:, 62 - 2 * i:190 - 2 * i],
                            rhs=e_tiles[(S, i)],
                            start=(i == 0),
                            stop=(i == NI - 1),
                        )

                def emit_recip(S):
                    rd32 = rdp.tile([128, 512], F32, tag="rd32", name="rd32")
                    nc.vector.reciprocal_approx_fast(out=rd32, in_=d_ps.pop(S))
                    rd = rdp.tile([128, 512], BF16, tag="rd", name="rd")
                    nc.vector.tensor_copy(rd, rd32)
                    rd_sb[S] = rd

                def emit_final(S):
                    ah = att_h.pop(S)
                    attT = attbp.tile([128, 2, 512], BF16, tag="attT", name="attT")
                    nc.scalar.add(attT[:, 0, :], ah[0], bv64_sb[:, 0:1])
                    nc.scalar.add(attT[:, 1, :], ah[1], bv64_sb[:, 1:2])
                    out_t = obufp.tile([128, 2, 512], BF16, tag="out_t", name="out_t")
                    for g in range(2):
                        psf = ps_fin.tile([128, 512], F32, tag="fin", name="psf")
                        for h in range(2):
                            nc.tensor.matmul(
                                psf,
                                lhsT=wcr_sb[:, h, g, :],
                                rhs=attT[:, h, :],
                                start=(h == 0),
                                stop=(h == 1),
                            )
                        nc.scalar.add(out_t[:, g, :], psf, bcr_sb[:, g:g + 1])
                        nc.sync.dma_start(
                            out=out_d[g * 128:(g + 1) * 128, S * 512:(S + 1) * 512],
                            in_=out_t[:, g, :],
                        )

                es_live = {}   # (S, t) -> es tile pair from emit_bc2

                def emit_v(p):
                    # V^T for blocks (2p, 2p+1), interleaved into S0 (ScalarE-
                    # bound there, so these PE/DVE ops ride along free).
                    # Borrows the bc0/bc1 PSUM banks, idle until S1.
                    psv = ps_bc.tile([128, 512], F32, tag=f"bc{p % 2}", name="psv")
                    for bb in range(2):
                        i = 2 * p + bb
                        for k in range(4):
                            nc.tensor.matmul(
                                psv[:, bb * 256:(bb + 1) * 256],
                                lhsT=x_sb[k][:, i * 128:(i + 1) * 128],
                                rhs=wvt_sb[:, k, :],
                                start=(k == 0),
                                stop=(k == 3),
                            )
                    nc.vector.tensor_copy(vt_sb[:, 2 * p:2 * p + 2, :], psv)

                def emit_bc2(S, t):
                    pair = []
                    for w, blk in ((0, t), (1, t + 16)):
                        ps = ps_bc.tile([128, 512], F32, tag=f"bc{w}", name=f"bc{w}")
                        nc.tensor.matmul(
                            ps,
                            lhsT=g128_sb[64 * w:64 * w + 64, t, :],
                            rhs=rd_sb[S][64 * w:64 * w + 64, :],
                            start=True,
                            stop=True,
                        )
                        es = esp.tile([128, 512], BF16, tag=f"es{w}", name=f"es{w}")
                        with nc.allow_low_precision(reason="attn weights are bf16 by design"):
                            nc.vector.tensor_mul(es, e_tiles[(S, blk)], ps)
                        pair.append(es)
                    es_live[(S, t)] = pair

                def emit_av2(S, t):
                    if S not in att_h:
                        att_h[S] = [
                            ps_att.tile([128, 512], F32, tag=f"att{h}", name=f"att{h}")
                            for h in range(2)
                        ]
                    pair = es_live.pop((S, t))
                    for w, blk in ((0, t), (1, t + 16)):
                        for h in range(2):
                            nc.tensor.matmul(
                                att_h[S][h],
                                lhsT=vt_sb[:, blk, h * 128:(h + 1) * 128],
                                rhs=pair[w],
                                start=(t == 0 and w == 0),
                                stop=(t == NP - 1 and w == 1),
                            )

                # Software pipeline: consumer side (bc/av/final of S-1) trails
                # the producer side (scores/dsum of S) far enough that the
                # reciprocal latency at each super-block boundary is covered.
                for S in range(NSUP + 1):
                    prod = S < NSUP
                    cons = S >= 1
                    for p in range(NP):
                        if prod:
                            emit_scores(S, p)
                            if p >= 1:
                                emit_dsum(S, p - 1)
                        if S == 0:
                            emit_v(p)
                        if cons and p >= 1:
                            emit_bc2(S - 1, p - 1)
                        if cons and p >= 2:
                            emit_av2(S - 1, p - 2)
                    if prod:
                        emit_dsum(S, NP - 1)
                        emit_recip(S)
                    if cons:
                        emit_bc2(S - 1, NP - 1)
                        emit_av2(S - 1, NP - 2)
                        emit_av2(S - 1, NP - 1)
                        emit_final(S - 1)
    nc.compile()
    return nc


def get_nc():
    if "nc" not in _CACHE:
        _CACHE["nc"] = _build_nc()
    return _CACHE["nc"]


def make_in_maps(inputs):
    rgb = np.asarray(inputs["rgb_features"], np.float32)
    chm = np.asarray(inputs["chm_features"], np.float32)
    Wq = np.asarray(inputs["Wq"], np.float32)
    bq = np.asarray(inputs["bq"], np.float32)
    Wk = np.asarray(inputs["Wk"], np.float32)
    bk = np.asarray(inputs["bk"], np.float32)
    Wv = np.asarray(inputs["Wv"], np.float32)
    bv = np.asarray(inputs["bv"], np.float32)
    Wcr = np.asarray(inputs["Wcr"], np.float32)
    bcr = np.asarray(inputs["bcr"], np.float32)

    Wqk = np.concatenate([Wq, Wk], axis=0)  # [128, 512]
    wqk = np.ascontiguousarray(Wqk.T.reshape(4, 128, 128).transpose(1, 0, 2)).astype(ml_dtypes.bfloat16)
    wvt = np.ascontiguousarray(Wv.T.reshape(4, 128, 256).transpose(1, 0, 2)).astype(ml_dtypes.bfloat16)
    wcr = np.ascontiguousarray(Wcr.T.reshape(2, 128, 2, 128).transpose(1, 0, 2, 3)).astype(ml_dtypes.bfloat16)
    bqk = np.ascontiguousarray(np.concatenate([bq, bk]).reshape(128, 1))
    bv64 = np.ascontiguousarray((64.0 * bv).reshape(2, 128).T)
    bcr2 = np.ascontiguousarray(bcr.reshape(2, 128).T)

    # dsum selector, shift-base form: slice for block i is g64b[:, 62-2i:190-2i]
    # with ones at u = 62+p//64 (d rows 2i+p//64) and u = 126+p//64 (dup rows
    # 64+2i+p//64, keeping every d_ps row finite for the reciprocal).
    p_ix = np.arange(128)
    g64b = np.zeros((128, 190), ml_dtypes.bfloat16)
    g64b[p_ix, 62 + p_ix // 64] = 1
    g64b[p_ix, 126 + p_ix // 64] = 1

    # row-packed 1/d broadcast selectors: pair t -> blocks (t, t+16).
    # window0 (partitions 0:64) serves block t from rd rows 2t + m//64;
    # window1 (partitions 64:128) serves block t+16 from the dup rows.
    g128p = np.zeros((128, 16, 128), ml_dtypes.bfloat16)
    m_ix = np.arange(128)
    for t in range(16):
        g128p[2 * t + m_ix // 64, t, m_ix] = 1
        g128p[64 + 2 * t + 32 + m_ix // 64, t, m_ix] = 1

    in_maps = []
    for core in range(8):
        b, par = divmod(core, 2)
        x = np.concatenate([rgb[b], chm[b]], axis=0).reshape(CIN, HW)
        if par:
            x = np.roll(x, -QCOLS, axis=1)
        x = np.ascontiguousarray(x)
        in_maps.append(
            {
                "x": x.astype(ml_dtypes.bfloat16),
                "wqk": wqk,
                "wvt": wvt,
                "wcr": wcr,
                "bqk": bqk,
                "bv64": bv64,
                "bcr2": bcr2,
                "g64b": g64b,
                "g128p": g128p,
            }
        )
    return in_maps


def assemble(outs):
    full = np.empty((B, C, HW), np.float32)
    for core in range(8):
        b, par = divmod(core, 2)
        full[b, :, par * QCOLS:(par + 1) * QCOLS] = np.asarray(outs[core], np.float32)
    return full.reshape(B, C, H, W)


def kernel(**inputs):
    from concourse.bass_utils import run_bass_kernel_spmd

    nc = get_nc()
    res = run_bass_kernel_spmd(nc, make_in_maps(inputs), core_ids=list(range(8)))
    return assemble([r["out"] for r in res.results])


# revision 15
# speedup vs baseline: 1.1588x; 1.0381x over previous
"""Cross-attention-concat kernel for Trainium2 (8 NeuronCores, Bass/Tile).

Math (per batch b):
  x   = concat(rgb, chm) on channels           [512, 4096]   (pixels hw = h*64+w)
  Q   = Wq x + bq ; K = Wk x + bk              [64, ...]
  V   = Wv x + bv                              [256, 4096]
  S   = Q^T K                                  [2048 hw, 4096 xy]
  A   = softmax over y within each x-group of 64 keys
  out = Wcr (A V^T)^T + bcr                    [256, 2048]

Sharding: core = (batch, H-half). The host rolls each batch's pixel axis by
2048*(core%2) so every core runs the same program with its queries at
columns 0:2048 of the rolled image.

Key structure (v2):
- Transposed-scores formulation: S^T [xy, q] per 128-xy block, exp on
  ScalarE, per-x-group softmax denominators collected by a selector matmul
  (g64), reciprocal broadcast back by a second selector matmul (g128),
  scale fused into the PSUM->SBUF pass feeding A@V.
- The two K=64-contraction matmuls (scores: c=64; 1/d broadcast: rd rows)
  are ROW-PACKED: two concurrent matmuls on PE row-halves via
  tile_position, halving their stream time. K blocks are packed
  even/odd into partition halves (k2), Q is duplicated into both halves,
  rd carries d duplicated into both halves (via g64's dup columns).
- Preamble is DMA-pipelined column-chunk-wise with a fused [Wq|Wk]
  stationary; Q/K land via small SBUF->SBUF shuffle DMAs.
- Reciprocal uses the fast approx DVE op and is emitted one pipeline
  stage early so it never stalls the PE.
"""

import numpy as np
import ml_dtypes

B, C, H, W = 4, 256, 64, 64
HW = H * W               # 4096
CIN = 2 * C              # 512
QCOLS = HW // 2          # 2048 queries per core
NSUP = QCOLS // 512      # 4 super-blocks of 512 queries
NI = HW // 128           # 32 xy-blocks of 128
NP = NI // 2             # 16 block-pairs

_CACHE = {}


def _build_nc():
    import concourse.bacc as bacc
    import concourse.tile as tile
    from concourse import mybir

    F32 = mybir.dt.float32
    BF16 = mybir.dt.bfloat16
    AF = mybir.ActivationFunctionType

    nc = bacc.Bacc("TRN2", target_bir_lowering=False, debug=False, num_devices=8)

    x_d = nc.dram_tensor("x", [CIN, HW], BF16, kind="ExternalInput").ap()
    g64_d = nc.dram_tensor("g64b", [128, 190], BF16, kind="ExternalInput").ap()
    g128_d = nc.dram_tensor("g128p", [128, 16, 128], BF16, kind="ExternalInput").ap()
    wqk_d = nc.dram_tensor("wqk", [128, 4, 128], BF16, kind="ExternalInput").ap()
    wvt_d = nc.dram_tensor("wvt", [128, 4, 256], BF16, kind="ExternalInput").ap()
    wcr_d = nc.dram_tensor("wcr", [128, 2, 2, 128], BF16, kind="ExternalInput").ap()
    bqk_d = nc.dram_tensor("bqk", [128, 1], F32, kind="ExternalInput").ap()
    bv64_d = nc.dram_tensor("bv64", [128, 2], F32, kind="ExternalInput").ap()
    bcr_d = nc.dram_tensor("bcr2", [128, 2], F32, kind="ExternalInput").ap()
    out_d = nc.dram_tensor("out", [C, QCOLS], BF16, kind="ExternalOutput").ap()

    with tile.TileContext(nc) as tc:
        with (
            tc.tile_pool(name="const", bufs=1) as constp,
            tc.tile_pool(name="qkv", bufs=1) as qkvp,
            tc.tile_pool(name="pbuf", bufs=40) as pbufp,
            tc.tile_pool(name="esbuf", bufs=2) as esp,
            tc.tile_pool(name="rdbuf", bufs=2) as rdp,
            tc.tile_pool(name="attbuf", bufs=2) as attbp,
            tc.tile_pool(name="obuf", bufs=2) as obufp,
        ):
            # ---- constants (weights first; big selector consts after x) ----
            wqk_sb = constp.tile([128, 4, 128], BF16)
            bqk_sb = constp.tile([128, 1], F32)
            nc.sync.dma_start(out=wqk_sb, in_=wqk_d)
            nc.sync.dma_start(out=bqk_sb, in_=bqk_d)

            q_sb = qkvp.tile([128, QCOLS], BF16)       # Q dup'd in both halves
            k2_sb = qkvp.tile([128, NP, 128], BF16)    # K blocks even/odd packed
            vt_sb = qkvp.tile([128, NI, 256], BF16)    # V^T [xy-block, 128, 256]

            wvt_sb = constp.tile([128, 4, 256], BF16)
            wcr_sb = constp.tile([128, 2, 2, 128], BF16)
            bv64_sb = constp.tile([128, 2], F32)
            bcr_sb = constp.tile([128, 2], F32)
            g64_sb = constp.tile([128, 190], BF16)
            g128_sb = constp.tile([128, 16, 128], BF16)

            # ---- preamble: 2 big x chunks, warm-up MMs during the DMA head,
            # QK projection only (V is interleaved into S0 below) ----
            x_sb = [qkvp.tile([128, HW], BF16, tag=f"x{k}", name=f"x{k}") for k in range(4)]
            # qkb[:, j, a, b, :] : col 512j+256a+128b; rows 0:64 = Q, 64:128 = K
            qkb = qkvp.tile([128, 8, 2, 2, 128], BF16, name="qkb")
            with tc.tile_pool(name="ps_pre", bufs=1, space="PSUM") as ps_pre:
                # warm-up MMs: trip the PE HAM throttle to full clock and keep
                # it busy while the x chunks stream in
                junk = ps_pre.tile([128, 512], F32, tag="junk", name="junk")
                for _ in range(20):
                    nc.tensor.matmul(junk, lhsT=wqk_sb[:, 0, :], rhs=wqk_sb[:, :, :], start=True, stop=True)
                # x descriptors split across the two HWDGE engines (Sync +
                # Scalar) so descriptor generation isn't serialized
                for half in range(2):
                    sl = slice(half * 2048, (half + 1) * 2048)
                    for k in range(4):
                        eng = nc.sync if k < 2 else nc.scalar
                        eng.dma_start(out=x_sb[k][:, sl], in_=x_d[k * 128:(k + 1) * 128, sl])
                nc.scalar.dma_start(out=wvt_sb, in_=wvt_d)
                nc.sync.dma_start(out=g64_sb, in_=g64_d)
                nc.scalar.dma_start(out=g128_sb, in_=g128_d)
                nc.sync.dma_start(out=wcr_sb, in_=wcr_d)
                nc.sync.dma_start(out=bv64_sb, in_=bv64_d)
                nc.sync.dma_start(out=bcr_sb, in_=bcr_d)

                for j in range(8):
                    sl = slice(j * 512, (j + 1) * 512)
                    psqk = ps_pre.tile([128, 512], F32, tag="psqk", bufs=2, name="psqk")
                    for k in range(4):
                        nc.tensor.matmul(
                            psqk,
                            lhsT=wqk_sb[:, k, :],
                            rhs=x_sb[k][:, sl],
                            start=(k == 0),
                            stop=(k == 3),
                        )
                    if j % 2 == 0:
                        nc.scalar.add(qkb[:, j], psqk, bqk_sb)
                    else:
                        nc.vector.tensor_scalar_add(qkb[:, j], psqk, bqk_sb)
                    if j == 3:
                        # Q dup: both partition halves get the same biased Q
                        nc.sync.dma_start(out=q_sb[0:64, :], in_=qkb[0:64, 0:4])
                        nc.sync.dma_start(out=q_sb[64:128, :], in_=qkb[0:64, 0:4])
                # K shuffle: even xy-blocks -> window0, odd -> window1
                nc.sync.dma_start(out=k2_sb[0:64, :, :], in_=qkb[64:128, :, :, 0, :])
                nc.sync.dma_start(out=k2_sb[64:128, :, :], in_=qkb[64:128, :, :, 1, :])

            # ---- main loop ----
            with (
                tc.tile_pool(name="ps_sc", bufs=1, space="PSUM") as ps_sc,
                tc.tile_pool(name="ps_d", bufs=1, space="PSUM") as ps_d,
                tc.tile_pool(name="ps_bc", bufs=1, space="PSUM") as ps_bc,
                tc.tile_pool(name="ps_att", bufs=1, space="PSUM") as ps_att,
                tc.tile_pool(name="ps_fin", bufs=1, space="PSUM") as ps_fin,
            ):
                e_tiles = {}   # (S, i) -> exp(S^T) tile, bf16 [128, 512]
                d_ps = {}      # S -> PSUM [128, 512] per-x-group sums (dup'd halves)
                rd_sb = {}     # S -> SBUF bf16 [128, 512] reciprocal sums
                att_h = {}     # S -> accumulating PSUM pair

                def emit_scores(S, p):
                    # pair p -> xy-blocks (2p, 2p+1) on PE row-halves
                    q0 = S * 512
                    for w in range(2):
                        ps = ps_sc.tile([128, 512], F32, tag=f"sc{w}", name=f"sc{w}")
                        nc.tensor.matmul(
                            ps,
                            lhsT=k2_sb[64 * w:64 * w + 64, p, :],
                            rhs=q_sb[64 * w:64 * w + 64, q0:q0 + 512],
                            start=True,
                            stop=True,
                        )
                        e = pbufp.tile([128, 512], BF16, tag="e", name="e")
                        nc.scalar.activation(e, ps, AF.Exp)
                        e_tiles[(S, 2 * p + w)] = e

                def emit_dsum(S, p):
                    # blocks (2p, 2p+1): scatter per-half sums into d rows
                    if S not in d_ps:
                        d_ps[S] = ps_d.tile([128, 512], F32, tag="d", name="d_ps")
                    for i in (2 * p, 2 * p + 1):
                        nc.tensor.matmul(
                            d_ps[S],
                            lhsT=g64_sb[:, 62 - 2 * i:190 - 2 * i],
                            rhs=e_tiles[(S, i)],
                            start=(i == 0),
                            stop=(i == NI - 1),
                        )

                def emit_recip(S):
                    rd32 = rdp.tile([128, 512], F32, tag="rd32", name="rd32")
                    nc.vector.reciprocal_approx_fast(out=rd32, in_=d_ps.pop(S))
                    rd = rdp.tile([128, 512], BF16, tag="rd", name="rd")
                    nc.scalar.copy(rd, rd32)
                    rd_sb[S] = rd

                def emit_final(S):
                    ah = att_h.pop(S)
                    attT = attbp.tile([128, 2, 512], BF16, tag="attT", name="attT")
                    nc.scalar.add(attT[:, 0, :], ah[0], bv64_sb[:, 0:1])
                    nc.scalar.add(attT[:, 1, :], ah[1], bv64_sb[:, 1:2])
                    out_t = obufp.tile([128, 2, 512], BF16, tag="out_t", name="out_t")
                    for g in range(2):
                        psf = ps_fin.tile([128, 512], F32, tag="fin", name="psf")
                        for h in range(2):
                            nc.tensor.matmul(
                                psf,
                                lhsT=wcr_sb[:, h, g, :],
                                rhs=attT[:, h, :],
                                start=(h == 0),
                                stop=(h == 1),
                            )
                        nc.scalar.add(out_t[:, g, :], psf, bcr_sb[:, g:g + 1])
                        nc.sync.dma_start(
                            out=out_d[g * 128:(g + 1) * 128, S * 512:(S + 1) * 512],
                            in_=out_t[:, g, :],
                        )

                es_live = {}   # (S, t) -> es tile pair from emit_bc2

                def emit_v(p):
                    # V^T for blocks (2p, 2p+1), interleaved into S0 (ScalarE-
                    # bound there, so these PE/DVE ops ride along free).
                    # Borrows the bc0/bc1 PSUM banks, idle until S1.
                    psv = ps_bc.tile([128, 512], F32, tag=f"bc{p % 2}", name="psv")
                    for bb in range(2):
                        i = 2 * p + bb
                        for k in range(4):
                            nc.tensor.matmul(
                                psv[:, bb * 256:(bb + 1) * 256],
                                lhsT=x_sb[k][:, i * 128:(i + 1) * 128],
                                rhs=wvt_sb[:, k, :],
                                start=(k == 0),
                                stop=(k == 3),
                            )
                    nc.vector.tensor_copy(vt_sb[:, 2 * p:2 * p + 2, :], psv)

                def emit_bc2(S, t, drain=False):
                    pair = []
                    for w, blk in ((0, t), (1, t + 16)):
                        ps = ps_bc.tile([128, 512], F32, tag=f"bc{w}", name=f"bc{w}")
                        nc.tensor.matmul(
                            ps,
                            lhsT=g128_sb[64 * w:64 * w + 64, t, :],
                            rhs=rd_sb[S][64 * w:64 * w + 64, :],
                            start=True,
                            stop=True,
                        )
                        es = esp.tile([128, 512], BF16, tag=f"es{w}", name=f"es{w}")
                        with nc.allow_low_precision(reason="attn weights are bf16 by design"):
                            nc.vector.tensor_mul(es, e_tiles[(S, blk)], ps)
                        pair.append(es)
                    es_live[(S, t)] = pair

                def emit_av2(S, t):
                    if S not in att_h:
                        att_h[S] = [
                            ps_att.tile([128, 512], F32, tag=f"att{h}", name=f"att{h}")
                            for h in range(2)
                        ]
                    pair = es_live.pop((S, t))
                    for w, blk in ((0, t), (1, t + 16)):
                        for h in range(2):
                            nc.tensor.matmul(
                                att_h[S][h],
                                lhsT=vt_sb[:, blk, h * 128:(h + 1) * 128],
                                rhs=pair[w],
                                start=(t == 0 and w == 0),
                                stop=(t == NP - 1 and w == 1),
                            )

                # Software pipeline: consumer side (bc/av/final of S-1) trails
                # the producer side (scores/dsum of S) far enough that the
                # reciprocal latency at each super-block boundary is covered.
                for S in range(NSUP + 1):
                    prod = S < NSUP
                    cons = S >= 1
                    drain = not prod
                    for p in range(NP):
                        if prod:
                            emit_scores(S, p)
                        # bc pair right after scores pair: the tiled (half-
                        # array) matmuls cluster so fewer LDW serialization
                        # boundaries are paid per iteration
                        if cons and p >= 1:
                            emit_bc2(S - 1, p - 1, drain)
                        if prod and p >= 1:
                            emit_dsum(S, p - 1)
                        if S == 0:
                            emit_v(p)
                        if cons and p >= 2:
                            emit_av2(S - 1, p - 2)
                    if prod:
                        emit_dsum(S, NP - 1)
                        emit_recip(S)
                    if cons:
                        emit_bc2(S - 1, NP - 1, drain)
                        emit_av2(S - 1, NP - 2)
                        emit_av2(S - 1, NP - 1)
                        emit_final(S - 1)
    nc.compile()
    return nc


def get_nc():
    if "nc" not in _CACHE:
        _CACHE["nc"] = _build_nc()
    return _CACHE["nc"]


def make_in_maps(inputs):
    rgb = np.asarray(inputs["rgb_features"], np.float32)
    chm = np.asarray(inputs["chm_features"], np.float32)
    Wq = np.asarray(inputs["Wq"], np.float32)
    bq = np.asarray(inputs["bq"], np.float32)
    Wk = np.asarray(inputs["Wk"], np.float32)
    bk = np.asarray(inputs["bk"], np.float32)
    Wv = np.asarray(inputs["Wv"], np.float32)
    bv = np.asarray(inputs["bv"], np.float32)
    Wcr = np.asarray(inputs["Wcr"], np.float32)
    bcr = np.asarray(inputs["bcr"], np.float32)

    Wqk = np.concatenate([Wq, Wk], axis=0)  # [128, 512]
    wqk = np.ascontiguousarray(Wqk.T.reshape(4, 128, 128).transpose(1, 0, 2)).astype(ml_dtypes.bfloat16)
    wvt = np.ascontiguousarray(Wv.T.reshape(4, 128, 256).transpose(1, 0, 2)).astype(ml_dtypes.bfloat16)
    wcr = np.ascontiguousarray(Wcr.T.reshape(2, 128, 2, 128).transpose(1, 0, 2, 3)).astype(ml_dtypes.bfloat16)
    bqk = np.ascontiguousarray(np.concatenate([bq, bk]).reshape(128, 1))
    bv64 = np.ascontiguousarray((64.0 * bv).reshape(2, 128).T)
    bcr2 = np.ascontiguousarray(bcr.reshape(2, 128).T)

    # dsum selector, shift-base form: slice for block i is g64b[:, 62-2i:190-2i]
    # with ones at u = 62+p//64 (d rows 2i+p//64) and u = 126+p//64 (dup rows
    # 64+2i+p//64, keeping every d_ps row finite for the reciprocal).
    p_ix = np.arange(128)
    g64b = np.zeros((128, 190), ml_dtypes.bfloat16)
    g64b[p_ix, 62 + p_ix // 64] = 1
    g64b[p_ix, 126 + p_ix // 64] = 1

    # row-packed 1/d broadcast selectors: pair t -> blocks (t, t+16).
    # window0 (partitions 0:64) serves block t from rd rows 2t + m//64;
    # window1 (partitions 64:128) serves block t+16 from the dup rows.
    g128p = np.zeros((128, 16, 128), ml_dtypes.bfloat16)
    m_ix = np.arange(128)
    for t in range(16):
        g128p[2 * t + m_ix // 64, t, m_ix] = 1
        g128p[64 + 2 * t + 32 + m_ix // 64, t, m_ix] = 1

    in_maps = []
    for core in range(8):
        b, par = divmod(core, 2)
        x = np.concatenate([rgb[b], chm[b]], axis=0).reshape(CIN, HW)
        if par:
            x = np.roll(x, -QCOLS, axis=1)
        x = np.ascontiguousarray(x)
        in_maps.append(
            {
                "x": x.astype(ml_dtypes.bfloat16),
                "wqk": wqk,
                "wvt": wvt,
                "wcr": wcr,
                "bqk": bqk,
                "bv64": bv64,
                "bcr2": bcr2,
                "g64b": g64b,
                "g128p": g128p,
            }
        )
    return in_maps


def assemble(outs):
    full = np.empty((B, C, HW), np.float32)
    for core in range(8):
        b, par = divmod(core, 2)
        full[b, :, par * QCOLS:(par + 1) * QCOLS] = np.asarray(outs[core], np.float32)
    return full.reshape(B, C, H, W)


def kernel(**inputs):
    from concourse.bass_utils import run_bass_kernel_spmd

    nc = get_nc()
    res = run_bass_kernel_spmd(nc, make_in_maps(inputs), core_ids=list(range(8)))
    return assemble([r["out"] for r in res.results])
